# revision 23
# baseline (speedup 1.0000x reference)
"""Trainium2 Bass kernel for CSSrcMapper (color-coded class map -> feature map).

Semantics (matches reference):
    d[b,c,h,w]  = floor(src[b,c,h,w] * 127.5 + 127.5)            (int color decode)
    match[b,k,h,w] = all_c(d[b,c,h,w] == colors[k,c])            (one-hot class)
    out[b,:,h,w] = sum_k match[b,k,h,w] * feats[k,:]             (feature scatter)

Strategy: data-parallel over 8 cores, shard = (batch, H-half).  Per core:
decode -> partition-broadcast channels over 19 class rows -> integer
equality vs color table -> AND -> one-hot [19,T] bf16 -> PE matmul with
hi/lo bf16 split of feats (exact to ~1e-5) -> PSUM -> SBUF -> HBM.
The kernel is HBM-write-bound: 128 MiB of f32 output per core.
"""

import os
from contextlib import ExitStack

import numpy as np
import ml_dtypes

import concourse.bass as bass
import concourse.mybir as mybir
import concourse.tile as tile
from concourse import bacc
from concourse.bass_utils import run_bass_kernel_spmd

B, H, W = 4, 256, 256
K = 19
FEAT = 1024
NCORES = 8
HSH = H // 2              # 128 rows per shard
NPIX = HSH * W            # 32768 pixels per core
TM = 2048                 # pixels per macro-tile
NCHUNK = FEAT // 128      # 8 output-channel chunks
SCALE = 127.5
BIAS = 383.5              # 127.5 + 256 -> decode lands at c + 256.5 exactly
OFF = 256                 # colors compared as colors + OFF (+0.5 center)

f32 = mybir.dt.float32
i32 = mybir.dt.int32
bf16 = mybir.dt.bfloat16


def _build_nc(npix=NPIX, tm=TM):
    nmt = npix // tm
    nc = bacc.Bacc("TRN2", target_bir_lowering=False, debug=False)
    src = nc.dram_tensor("src", [3, npix], f32, kind="ExternalInput").ap()
    cols = nc.dram_tensor("cols", [96, 1], f32, kind="ExternalInput").ap()
    w127 = nc.dram_tensor("w127", [3, 96], f32, kind="ExternalInput").ap()
    sel = nc.dram_tensor("sel", [96, K], bf16, kind="ExternalInput").ap()
    fhi = nc.dram_tensor("fhi", [K, FEAT], bf16, kind="ExternalInput").ap()
    flo = nc.dram_tensor("flo", [K, FEAT], bf16, kind="ExternalInput").ap()
    out = nc.dram_tensor("out", [FEAT, npix], f32, kind="ExternalOutput").ap()

    with tile.TileContext(nc) as tc, ExitStack() as ctx:
        const_p = ctx.enter_context(tc.tile_pool(name="const", bufs=1))
        src_p = ctx.enter_context(tc.tile_pool(name="srcp", bufs=3))
        dec_p = ctx.enter_context(tc.tile_pool(name="decp", bufs=2))
        rep_p = ctx.enter_context(tc.tile_pool(name="repp", bufs=2, space="PSUM"))
        eq_p = ctx.enter_context(tc.tile_pool(name="eqp", bufs=2))
        and_p = ctx.enter_context(tc.tile_pool(name="andp", bufs=2, space="PSUM"))
        match_p = ctx.enter_context(tc.tile_pool(name="matchp", bufs=2))
        out_p = ctx.enter_context(tc.tile_pool(name="outp", bufs=4))
        psum_p = ctx.enter_context(tc.tile_pool(name="psum", bufs=2, space="PSUM"))

        colst = const_p.tile([96, 1], f32)
        nc.sync.dma_start(colst[:], cols[:])
        w127_sb = const_p.tile([3, 96], f32)
        nc.sync.dma_start(w127_sb[:], w127[:])
        sel_sb = const_p.tile([96, K], bf16)
        nc.sync.dma_start(sel_sb[:], sel[:])
        fhi_sb = const_p.tile([K, FEAT], bf16)
        nc.sync.dma_start(fhi_sb[:], fhi[:])
        flo_sb = const_p.tile([K, FEAT], bf16)
        nc.sync.dma_start(flo_sb[:], flo[:])

        for m in range(nmt):
            msl = slice(m * tm, (m + 1) * tm)
            s = src_p.tile([3, tm], f32)
            nc.sync.dma_start(s[:], src[:, msl])

            # PE broadcast: t[row,n] = 127.5 * s[group(row), n]  (fp32, exact
            # to ~2e-3 even if PE decomposes fp32 -- margin is 0.25)
            # then (t + (127-color))^2 < 0.25  <=>  decoded color matches.
            sq = eq_p.tile([96, tm], f32, tag="sq")
            for n in range(tm // 512):
                nsl = slice(n * 512, (n + 1) * 512)
                tps = rep_p.tile(
                    [96, 512], f32, space="PSUM", name=f"tps_{m}_{n}", tag="tps"
                )
                nc.tensor.matmul(
                    tps[:], w127_sb[:], s[:, nsl], start=True, stop=True
                )
                nc.scalar.activation(
                    sq[:, nsl], tps[:], mybir.ActivationFunctionType.Square,
                    bias=colst[:], scale=1.0,
                )
            eq = eq_p.tile([96, tm], bf16, tag="eq")
            nc.vector.tensor_scalar(
                eq[:], sq[:], 0.25, None, mybir.AluOpType.is_lt
            )

            # AND across the 3 channel groups via PE: sel sums each class's
            # three eq rows; sum == 3 <=> all channels matched (one-hot).
            match = match_p.tile([K, tm], bf16)
            for n in range(tm // 512):
                nsl = slice(n * 512, (n + 1) * 512)
                mps = and_p.tile([K, 512], f32, space="PSUM", name=f"mps_{m}_{n}", tag="mps")
                nc.tensor.matmul(
                    mps[:], sel_sb[:], eq[:, nsl], start=True, stop=True
                )
                nc.vector.tensor_scalar(
                    match[:, nsl], mps[:], 3.0, None, mybir.AluOpType.is_equal
                )

            # feats.T @ match per 128-channel chunk, hi/lo accumulation
            for j in range(NCHUNK):
                jsl = slice(j * 128, (j + 1) * 128)
                ob = out_p.tile([128, tm], f32)
                for hh in range(tm // 1024):
                    ps = psum_p.tile([128, 1024], f32, space="PSUM")
                    for q in range(2):
                        nsl = slice(hh * 1024 + q * 512, hh * 1024 + q * 512 + 512)
                        qsl = slice(q * 512, (q + 1) * 512)
                        nc.tensor.matmul(
                            ps[:, qsl], fhi_sb[:, jsl], match[:, nsl],
                            start=True, stop=False,
                        )
                    for q in range(2):
                        nsl = slice(hh * 1024 + q * 512, hh * 1024 + q * 512 + 512)
                        qsl = slice(q * 512, (q + 1) * 512)
                        nc.tensor.matmul(
                            ps[:, qsl], flo_sb[:, jsl], match[:, nsl],
                            start=False, stop=True,
                        )
                    osl = slice(hh * 1024, (hh + 1) * 1024)
                    if (j * (tm // 1024) + hh) % 2 == 0:
                        nc.scalar.copy(ob[:, osl], ps[:])
                    else:
                        nc.vector.tensor_copy(ob[:, osl], ps[:])
                nc.sync.dma_start(out[jsl, msl], ob[:])
    nc.compile()
    return nc


_CACHE = {}


def _get_nc():
    if "nc" not in _CACHE:
        _CACHE["nc"] = _build_nc()
    return _CACHE["nc"]


def _host_prep(src, colors, feats):
    src = np.asarray(src, dtype=np.float32)
    colors = np.asarray(colors, dtype=np.int32)
    feats = np.asarray(feats, dtype=np.float32)

    colstack = np.full((96, 1), 1e9, dtype=np.float32)
    for c in range(3):
        colstack[c * 32:c * 32 + K, 0] = 127.0 - colors[:, c].astype(np.float32)
    w127mat = np.zeros((3, 96), dtype=np.float32)
    for c in range(3):
        w127mat[c, c * 32:(c + 1) * 32] = 127.5
    selmat = np.zeros((96, K), dtype=ml_dtypes.bfloat16)
    for c in range(3):
        for k in range(K):
            selmat[c * 32 + k, k] = 1
    fhi = feats.astype(ml_dtypes.bfloat16)
    flo = (feats - fhi.astype(np.float32)).astype(ml_dtypes.bfloat16)

    in_maps = []
    for core in range(NCORES):
        b, half = divmod(core, 2)
        shard = np.ascontiguousarray(
            src[b, :, half * HSH:(half + 1) * HSH, :]
        ).reshape(3, NPIX)
        in_maps.append(
            {"src": shard, "cols": colstack, "w127": w127mat, "sel": selmat,
             "fhi": fhi, "flo": flo}
        )
    return in_maps


def _assemble(results):
    full = np.empty((B, FEAT, H, W), dtype=np.float32)
    for core in range(NCORES):
        b, half = divmod(core, 2)
        full[b, :, half * HSH:(half + 1) * HSH, :] = results[core]["out"].reshape(
            FEAT, HSH, W
        )
    return full


def kernel(src, colors, feats):
    nc = _get_nc()
    in_maps = _host_prep(src, colors, feats)
    res = run_bass_kernel_spmd(nc, in_maps, list(range(NCORES)))
    return _assemble(res.results)


# revision 24
# speedup vs baseline: 1.2038x; 1.2038x over previous
"""Trainium2 Bass kernel for CSSrcMapper (color-coded class map -> feature map).

Semantics (matches reference):
    d[b,c,h,w]  = floor(src[b,c,h,w] * 127.5 + 127.5)            (int color decode)
    match[b,k,h,w] = all_c(d[b,c,h,w] == colors[k,c])            (one-hot class)
    out[b,:,h,w] = sum_k match[b,k,h,w] * feats[k,:]             (feature scatter)

Strategy: data-parallel over 8 cores, shard = (batch, H-half).  Per core:
decode -> partition-broadcast channels over 19 class rows -> integer
equality vs color table -> AND -> one-hot [19,T] bf16 -> PE matmul with
hi/lo bf16 split of feats (exact to ~1e-5) -> PSUM -> SBUF -> HBM.
The kernel is HBM-write-bound: 128 MiB of f32 output per core.
"""

import os
from contextlib import ExitStack

import numpy as np
import ml_dtypes

import concourse.bass as bass
import concourse.mybir as mybir
import concourse.tile as tile
from concourse import bacc
from concourse.bass_utils import run_bass_kernel_spmd

B, H, W = 4, 256, 256
K = 19
FEAT = 1024
NCORES = 8
HSH = H // 2              # 128 rows per shard
NPIX = HSH * W            # 32768 pixels per core
TM = 2048                 # pixels per macro-tile
NCHUNK = FEAT // 128      # 8 output-channel chunks
SCALE = 127.5
BIAS = 383.5              # 127.5 + 256 -> decode lands at c + 256.5 exactly
OFF = 256                 # colors compared as colors + OFF (+0.5 center)

f32 = mybir.dt.float32
i32 = mybir.dt.int32
bf16 = mybir.dt.bfloat16


def _build_nc(npix=NPIX, tm=TM):
    nmt = npix // tm
    nc = bacc.Bacc("TRN2", target_bir_lowering=False, debug=False)
    src = nc.dram_tensor("src", [3, npix], f32, kind="ExternalInput").ap()
    cols = nc.dram_tensor("cols", [96, 1], f32, kind="ExternalInput").ap()
    w127 = nc.dram_tensor("w127", [3, 96], f32, kind="ExternalInput").ap()
    sel = nc.dram_tensor("sel", [96, 64], bf16, kind="ExternalInput").ap()
    fst = nc.dram_tensor("fst", [64, FEAT], bf16, kind="ExternalInput").ap()
    out = nc.dram_tensor("out", [FEAT, npix], f32, kind="ExternalOutput").ap()

    with tile.TileContext(nc) as tc, ExitStack() as ctx:
        const_p = ctx.enter_context(tc.tile_pool(name="const", bufs=1))
        src_p = ctx.enter_context(tc.tile_pool(name="srcp", bufs=3))
        dec_p = ctx.enter_context(tc.tile_pool(name="decp", bufs=2))
        rep_p = ctx.enter_context(tc.tile_pool(name="repp", bufs=2, space="PSUM"))
        eq_p = ctx.enter_context(tc.tile_pool(name="eqp", bufs=2))
        and_p = ctx.enter_context(tc.tile_pool(name="andp", bufs=2, space="PSUM"))
        match_p = ctx.enter_context(tc.tile_pool(name="matchp", bufs=2))
        out_p = ctx.enter_context(tc.tile_pool(name="outp", bufs=4))
        psum_p = ctx.enter_context(tc.tile_pool(name="psum", bufs=2, space="PSUM"))

        colst = const_p.tile([96, 1], f32)
        nc.sync.dma_start(colst[:], cols[:])
        w127_sb = const_p.tile([3, 96], f32)
        nc.sync.dma_start(w127_sb[:], w127[:])
        sel_sb = const_p.tile([96, 64], bf16)
        nc.sync.dma_start(sel_sb[:], sel[:])
        fst_sb = const_p.tile([64, FEAT], bf16)
        nc.sync.dma_start(fst_sb[:], fst[:])

        for m in range(nmt):
            msl = slice(m * tm, (m + 1) * tm)
            s = src_p.tile([3, tm], f32)
            nc.sync.dma_start(s[:], src[:, msl])

            # PE broadcast: t[row,n] = 127.5 * s[group(row), n]  (fp32),
            # then sq = (t + (127-color))^2 as bf16 (mismatch >= ~1, match
            # ~ 1e-8; bf16 rounding is harmless at that margin).
            sq = eq_p.tile([96, tm], bf16, tag="sq")
            for n in range(tm // 512):
                nsl = slice(n * 512, (n + 1) * 512)
                tps = rep_p.tile(
                    [96, 512], f32, space="PSUM", name=f"tps_{m}_{n}", tag="tps"
                )
                nc.tensor.matmul(
                    tps[:], w127_sb[:], s[:, nsl], start=True, stop=True
                )
                nc.scalar.activation(
                    sq[:, nsl], tps[:], mybir.ActivationFunctionType.Square,
                    bias=colst[:], scale=1.0,
                )

            # AND across channel groups on the PE: sel sums each class's three
            # squared distances into rows k and 32+k; sum < 0.25 <=> one-hot
            # match.  Rows 32..50 duplicate the match so one stacked hi/lo
            # matmul can do the full-precision lookup in a single pass.
            match = match_p.tile([64, tm], bf16)
            for n in range(tm // 512):
                nsl = slice(n * 512, (n + 1) * 512)
                mps = and_p.tile([64, 512], f32, space="PSUM", name=f"mps_{m}_{n}", tag="mps")
                nc.tensor.matmul(
                    mps[:], sel_sb[:], sq[:, nsl], start=True, stop=True
                )
                nc.vector.tensor_scalar(
                    match[:, nsl], mps[:], 0.25, None, mybir.AluOpType.is_lt
                )

            # stacked hi/lo lookup: one matmul per 512-pixel slice per chunk
            # (rows 0..18 hi-feats hit match rows 0..18, rows 32..50 lo-feats
            # hit the duplicated match rows -- the array sums hi+lo exactly).
            for j in range(NCHUNK):
                jsl = slice(j * 128, (j + 1) * 128)
                ob = out_p.tile([128, tm], f32)
                for hh in range(tm // 1024):
                    ps = psum_p.tile([128, 1024], f32, space="PSUM")
                    for q in range(2):
                        nsl = slice(hh * 1024 + q * 512, hh * 1024 + q * 512 + 512)
                        qsl = slice(q * 512, (q + 1) * 512)
                        nc.tensor.matmul(
                            ps[:, qsl], fst_sb[:, jsl], match[:, nsl],
                            start=True, stop=True,
                        )
                    osl = slice(hh * 1024, (hh + 1) * 1024)
                    if (j * (tm // 1024) + hh) % 2 == 0:
                        nc.scalar.copy(ob[:, osl], ps[:])
                    else:
                        nc.vector.tensor_copy(ob[:, osl], ps[:])
                nc.sync.dma_start(out[jsl, msl], ob[:])
    nc.compile()
    return nc


_CACHE = {}


def _get_nc():
    if "nc" not in _CACHE:
        _CACHE["nc"] = _build_nc()
    return _CACHE["nc"]


def _host_prep(src, colors, feats):
    src = np.asarray(src, dtype=np.float32)
    colors = np.asarray(colors, dtype=np.int32)
    feats = np.asarray(feats, dtype=np.float32)

    colstack = np.full((96, 1), 1e9, dtype=np.float32)
    for c in range(3):
        colstack[c * 32:c * 32 + K, 0] = 127.0 - colors[:, c].astype(np.float32)
    w127mat = np.zeros((3, 96), dtype=np.float32)
    for c in range(3):
        w127mat[c, c * 32:(c + 1) * 32] = 127.5
    selmat = np.zeros((96, 64), dtype=ml_dtypes.bfloat16)
    for c in range(3):
        for k in range(K):
            selmat[c * 32 + k, k] = 1
            selmat[c * 32 + k, 32 + k] = 1
    fhi = feats.astype(ml_dtypes.bfloat16)
    flo = (feats - fhi.astype(np.float32)).astype(ml_dtypes.bfloat16)
    fstack = np.zeros((64, FEAT), dtype=ml_dtypes.bfloat16)
    fstack[0:K] = fhi
    fstack[32:32 + K] = flo

    in_maps = []
    for core in range(NCORES):
        b, half = divmod(core, 2)
        shard = np.ascontiguousarray(
            src[b, :, half * HSH:(half + 1) * HSH, :]
        ).reshape(3, NPIX)
        in_maps.append(
            {"src": shard, "cols": colstack, "w127": w127mat, "sel": selmat,
             "fst": fstack}
        )
    return in_maps


def _assemble(results):
    full = np.empty((B, FEAT, H, W), dtype=np.float32)
    for core in range(NCORES):
        b, half = divmod(core, 2)
        full[b, :, half * HSH:(half + 1) * HSH, :] = results[core]["out"].reshape(
            FEAT, HSH, W
        )
    return full


def kernel(src, colors, feats):
    nc = _get_nc()
    in_maps = _host_prep(src, colors, feats)
    res = run_bass_kernel_spmd(nc, in_maps, list(range(NCORES)))
    return _assemble(res.results)
